# revision 25
# baseline (speedup 1.0000x reference)
"""MLA-style attention (nn_Attention_7868380086611) on 8 TRN2 NeuronCores.

Strategy (v7)
-------------
Head-parallel attention (2 of 16 heads per core), query path fully absorbed
on the host (W_dq.T @ W_uq / W_dq.T @ W_qr.T), tiny shared kv latent
(c_kv 512 rows + k_r 64 rows) computed T-sharded and AllGathered once.

v7 over the 261us v3 baseline:
- Phase-1 inputs host-prepacked into SBUF-image layout: 9 simple 2D DMAs
  (cheap descriptor gen) interleaved so the c-outer 3-accumulator matmul
  chain starts as soon as the first quarter lands.  cc_in stores go on the
  SYNC queue *before* the 8MB xT loads (descriptor rings drain FIFO), so
  the AllGather triggers at ~26us instead of ~42us.
- Rope operands zero-padded 64->128 partitions (K=64 matmuls measured
  369ns vs 260ns for K=128).
- Causal diagonal-suffix: diagonal score blocks only compute the unmasked
  column suffix for score/rope/AV matmuls, exp, mask and accumulate.
- kT is l-outer (8 PSUM accumulators at once) right after the gather.
- Softmax denominator division done on the HOST (kernel returns
  unnormalized y [HPC*HS, T] + den [HPC, T]): kills all den/yT transposes.
- PSUM->SBUF projection copies on Vector (Scalar = exp critical path).
"""

import math
import sys

import numpy as np

sys.path.insert(0, "/opt/trn_rl_repo")

import ml_dtypes  # noqa: E402

from concourse import bacc, bass, masks, mybir  # noqa: E402
from concourse.bass_utils import run_bass_kernel_spmd  # noqa: E402
from concourse.tile import TileContext  # noqa: E402

B, T, C = 1, 2048, 2048
NH, HS = 16, 128
NLQ, NLKV, DHR = 1536, 512, 64
NCORES = 8
HPC = NH // NCORES          # heads per core = 2
TS = T // NCORES            # 256-token shard for the kv down-projection
P = 128
LKV = NLKV // P             # 4
CCH = C // P                # 16 c-chunks
TJ = T // 512               # 4 t-chunks of 512
SC = T // P                 # 16 s-chunks
SCALE = 1.0 / math.sqrt(HS + DHR)

BF = mybir.dt.bfloat16
F32 = mybir.dt.float32
Exp = mybir.ActivationFunctionType.Exp

GR = NLKV // 2 + DHR        # 320 rows in the all-gather buffer
TS2 = 2 * TS                # 512-token slice


def build_nc():
    nc = bacc.Bacc(None, target_bir_lowering=False, num_devices=NCORES)

    xTp = nc.declare_dram_parameter("xTp", [CCH, P, T], BF, isOutput=False)
    wdkvP = nc.declare_dram_parameter("wdkvP", [P, CCH * 256], BF,
                                      isOutput=False)
    wkrP = nc.declare_dram_parameter("wkrP", [P, CCH * DHR], BF,
                                     isOutput=False)
    cos2T = nc.declare_dram_parameter("cos2T", [DHR, T], BF, isOutput=False)
    sin2T = nc.declare_dram_parameter("sin2T", [DHR, T], BF, isOutput=False)
    wq = nc.declare_dram_parameter("wq", [CCH, P, HPC * HS], BF, isOutput=False)
    wqr = nc.declare_dram_parameter("wqr", [CCH, P, HPC * DHR], BF, isOutput=False)
    wukT = nc.declare_dram_parameter("wukT", [LKV, P, HPC * HS], BF, isOutput=False)
    bc = nc.declare_dram_parameter("bc", [LKV, P, HPC * HS], BF, isOutput=False)
    xsP = nc.declare_dram_parameter("xsP", [P, CCH * TS2], BF, isOutput=False)
    out = nc.declare_dram_parameter("out", [HPC * HS, T], F32, isOutput=True)
    out_den = nc.declare_dram_parameter("out_den", [HPC, T], F32, isOutput=True)

    cc_in_kv = nc.dram_tensor("cc_in_kv", [GR, TS2], BF)
    cc_out_kv = nc.dram_tensor("cc_out_kv", [NCORES, GR, TS2], BF,
                               addr_space="Shared")

    with TileContext(nc) as tc:
        with (
            tc.tile_pool(name="persist", bufs=1) as persist,
            tc.tile_pool(name="lat", bufs=1) as lat,
            tc.tile_pool(name="proj", bufs=1) as proj,
            tc.tile_pool(name="wts", bufs=1) as wts,
        ):
            # ---- constants (gpsimd: free early) ----
            junk = persist.tile([P, 512], BF)
            nc.gpsimd.memset(junk[:], 0.0)
            ones_bf = persist.tile([P, 1], BF)
            nc.gpsimd.memset(ones_bf[:], 1.0)
            # single [128,128] triangular mask: 1 iff t - s >= 0
            tri = persist.tile([P, P], BF)
            nc.gpsimd.memset(tri[:], 1.0)
            nc.gpsimd.affine_select(
                out=tri[:], in_=tri[:],
                compare_op=mybir.AluOpType.is_ge,
                fill=0.0, base=0, channel_multiplier=-1,
                pattern=[[1, P]],
            )
            cos_sb = persist.tile([DHR, T], BF)
            sin_sb = persist.tile([DHR, T], BF)

            # rope-padded tiles: rows 64-127 stay zero (K=128 matmuls)
            qr_rope = proj.tile([P, HPC * T], BF)
            kr_rope = proj.tile([P, T], BF)
            qr_h1 = proj.tile([P, T], BF)
            nc.vector.memset(qr_rope[:], 0.0)
            nc.vector.memset(kr_rope[:], 0.0)
            nc.vector.memset(qr_h1[:], 0.0)

            qT = proj.tile([P, HPC * T], BF)
            kT = proj.tile([P, HPC * T], BF)
            qr2 = proj.tile([P, T], BF)
            v_sb = proj.tile([P, SC * HPC * HS], BF)

            # ---- phase 1: c_kv^T/k_r^T for own slice -> AllGather ----
            with (
                tc.tile_pool(name="p1w", bufs=1) as p1w,
                tc.tile_pool(name="p1ps", bufs=1, space="PSUM") as p1ps,
                tc.tile_pool(name="p1sh", bufs=1) as p1sh,
            ):
                # PE warm-up while the first DMA quarters land (~3us)
                ps_w = p1ps.tile([P, 512], F32, name="ps_warm")
                for _ in range(5):
                    nc.tensor.matmul(ps_w[:], junk[:, 0:P], junk[:],
                                     start=True, stop=True)

                # interleaved quarter loads (host-prepacked, simple 2D
                # DMAs); per-quarter tiles so the c=0 matmuls only wait on
                # quarter 0, not the whole image
                wdkv_q = [p1w.tile([P, 1024], BF, name=f"wdkv{g}",
                                   tag=f"wdkv{g}") for g in range(4)]
                xs_q = [p1w.tile([P, 2048], BF, name=f"xs{g}",
                                 tag=f"xs{g}") for g in range(4)]
                wkr_sb = p1w.tile([P, CCH * DHR], BF, name="wkr_sb")
                for g in range(4):
                    nc.sync.dma_start(
                        wdkv_q[g][:], wdkvP.ap()[:, g * 1024:(g + 1) * 1024]
                    )
                    nc.sync.dma_start(
                        xs_q[g][:], xsP.ap()[:, g * 2048:(g + 1) * 2048]
                    )
                    if g == 0:
                        nc.sync.dma_start(wkr_sb[:], wkrP.ap())
                wqr_all = wts.tile([P, CCH * HPC * DHR], BF)
                nc.sync.dma_start(
                    wqr_all[:].rearrange("p (n m) -> p n m", n=CCH),
                    wqr.ap().rearrange("n p m -> p n m"),
                )

                def xstile(c):
                    return xs_q[c // 4][:, (c % 4) * TS2:(c % 4 + 1) * TS2]

                def wdkv_sl(c, ls):
                    return wdkv_q[c // 4][:, (c % 4) * 256 + ls * P:
                                          (c % 4) * 256 + (ls + 1) * P]

                ps0 = p1ps.tile([P, TS2], F32, name="ps0")
                ps1 = p1ps.tile([P, TS2], F32, name="ps1")
                pskr = p1ps.tile([DHR, TS2], F32, name="pskr")
                for c in range(CCH):
                    st, sp = (c == 0), (c == CCH - 1)
                    nc.tensor.matmul(ps0[:], wdkv_sl(c, 0), xstile(c),
                                     start=st, stop=sp)
                    nc.tensor.matmul(ps1[:], wdkv_sl(c, 1), xstile(c),
                                     start=st, stop=sp)
                    nc.tensor.matmul(pskr[:],
                                     wkr_sb[:, c * DHR:(c + 1) * DHR],
                                     xstile(c), start=st, stop=sp)
                sh0 = p1sh.tile([P, TS2], BF, name="sh0")
                sh1 = p1sh.tile([P, TS2], BF, name="sh1")
                shkr = p1sh.tile([DHR, TS2], BF, name="shkr")
                nc.scalar.copy(sh0[:], ps0[:])
                nc.vector.tensor_copy(sh1[:], ps1[:])
                nc.scalar.copy(shkr[:], pskr[:])
                # stores on the SYNC queue, BEFORE the big xT loads are
                # enqueued -> they reach the FIFO DMA rings first and the
                # AllGather triggers ~16us earlier.
                nc.sync.dma_start(cc_in_kv[0:P, :], sh0[:])
                nc.sync.dma_start(cc_in_kv[P:2 * P, :], sh1[:])
                nc.sync.dma_start(cc_in_kv[2 * P:GR, :], shkr[:])

                nc.gpsimd.collective_compute(
                    "AllGather",
                    mybir.AluOpType.bypass,
                    replica_groups=[list(range(NCORES))],
                    ins=[cc_in_kv.ap().opt()],
                    outs=[cc_out_kv.ap().opt()],
                )

                # keep the HAM clock gate open across the xT-load bubble
                for _ in range(6):
                    nc.tensor.matmul(ps_w[:], junk[:, 0:P], junk[:],
                                     start=True, stop=True)

            nc.scalar.dma_start(cos_sb[:], cos2T[:, :])
            nc.scalar.dma_start(sin_sb[:], sin2T[:, :])

            # ---- remaining loads (sync queue, after the cc stores) ----
            wq_all = wts.tile([P, CCH * HPC * HS], BF)
            nc.sync.dma_start(
                wq_all[:].rearrange("p (n m) -> p n m", n=CCH),
                wq.ap().rearrange("n p m -> p n m"),
            )
            xt = []
            for cgrp in range(4):
                t = lat.tile([P, 4 * T], BF, name=f"xt{cgrp}", tag=f"xt{cgrp}")
                for j in range(4):
                    nc.sync.dma_start(
                        t[:, j * T:(j + 1) * T], xTp.ap()[4 * cgrp + j]
                    )
                xt.append(t)

            def xtile(c):
                return xt[c // 4][:, (c % 4) * T:(c % 4 + 1) * T]

            wuk_all = wts.tile([P, LKV * HPC * HS], BF)
            nc.sync.dma_start(
                wuk_all[:].rearrange("p (n m) -> p n m", n=LKV),
                wukT.ap().rearrange("n p m -> p n m"),
            )
            b_all = wts.tile([P, LKV * HPC * HS], BF)
            nc.sync.dma_start(
                b_all[:].rearrange("p (n m) -> p n m", n=LKV),
                bc.ap().rearrange("n p m -> p n m"),
            )

            with tc.tile_pool(name="rtmp", bufs=1) as rtmp:

                def rope(dst, src):
                    # dst = src * [cos;cos] + swap_halves(src) * [-sin;sin]
                    sw = rtmp.tile([DHR, T], BF, name="rsw", tag="rsw")
                    nc.sync.dma_start(sw[0:32, :], src[32:64, :])
                    nc.sync.dma_start(sw[32:64, :], src[0:32, :])
                    ta = rtmp.tile([DHR, T], BF, name="rta", tag="rta")
                    tb = rtmp.tile([DHR, T], BF, name="rtb", tag="rtb")
                    nc.vector.tensor_mul(ta[:], src, cos_sb[:])
                    nc.vector.tensor_mul(tb[:], sw[:], sin_sb[:])
                    nc.vector.tensor_add(dst, ta[:], tb[:])

                with tc.tile_pool(name="p5ps", bufs=5, space="PSUM") as p5ps:
                    # q_r^T both heads in one pass (M=128), tj-inner
                    ps_qr = [
                        p5ps.tile([P, 512], F32, name=f"ps_qr{tj}", tag="p5")
                        for tj in range(TJ)
                    ]
                    for c in range(CCH):
                        for tj in range(TJ):
                            nc.tensor.matmul(
                                ps_qr[tj][:],
                                wqr_all[:, c * HPC * DHR:(c + 1) * HPC * DHR],
                                xtile(c)[:, tj * 512:(tj + 1) * 512],
                                start=(c == 0),
                                stop=(c == CCH - 1),
                            )
                    for tj in range(TJ):
                        nc.vector.tensor_copy(
                            qr2[:, tj * 512:(tj + 1) * 512], ps_qr[tj][:]
                        )
                    nc.sync.dma_start(qr_h1[0:DHR, :], qr2[DHR:P, :])
                    rope(qr_rope[0:DHR, 0:T], qr2[0:DHR, :])
                    rope(qr_rope[0:DHR, T:HPC * T], qr_h1[0:DHR, :])

                    # q^T per head, tj-inner
                    for h in range(HPC):
                        ps_q = [
                            p5ps.tile([P, 512], F32, name=f"ps_q{h}_{tj}",
                                      tag="p5")
                            for tj in range(TJ)
                        ]
                        for c in range(CCH):
                            for tj in range(TJ):
                                nc.tensor.matmul(
                                    ps_q[tj][:],
                                    wq_all[:, c * HPC * HS + h * HS:
                                           c * HPC * HS + (h + 1) * HS],
                                    xtile(c)[:, tj * 512:(tj + 1) * 512],
                                    start=(c == 0),
                                    stop=(c == CCH - 1),
                                )
                        for tj in range(TJ):
                            nc.vector.tensor_copy(
                                qT[:, h * T + tj * 512: h * T + (tj + 1) * 512],
                                ps_q[tj][:],
                            )

                # ---- gathered kv latents (rank r = 2*tb+hi holds nlkv-half
                # hi of token slice tb; kr lives on the even-rank halves) ----
                cc_halves = cc_out_kv.ap().rearrange(
                    "(b two) r u -> two b r u", two=2
                )
                ckv_t = [
                    lat.tile([P, T], BF, name=f"ckv{l}", tag=f"ckv{l}")
                    for l in range(LKV)
                ]
                # per-(l, block) simple 2D loads, l-major, split across the
                # two HWDGE queues: kT's l=0 matmuls start ~3us after the
                # mesh ends instead of ~14us (cheap descgen, no grouped-tile
                # wait)
                for l in range(LKV):
                    eng = nc.scalar if l % 2 == 0 else nc.sync
                    for tb in range(4):
                        eng.dma_start(
                            ckv_t[l][:, tb * 512:(tb + 1) * 512],
                            cc_out_kv.ap()[2 * tb + l // 2,
                                           (l % 2) * P:(l % 2 + 1) * P, :],
                        )
                kr_raw = lat.tile([DHR, T], BF)
                for tb in range(4):
                    nc.sync.dma_start(
                        kr_raw[:, tb * 512:(tb + 1) * 512],
                        cc_out_kv.ap()[2 * tb, 2 * P:GR, :],
                    )
                with tc.tile_pool(name="rtmp2", bufs=1) as rtmp2:
                    sw = rtmp2.tile([DHR, T], BF, name="krsw")
                    nc.sync.dma_start(sw[0:32, :], kr_raw[32:64, :])
                    nc.sync.dma_start(sw[32:64, :], kr_raw[0:32, :])
                    ta = rtmp2.tile([DHR, T], BF, name="krta")
                    tb2 = rtmp2.tile([DHR, T], BF, name="krtb")
                    nc.vector.tensor_mul(ta[:], kr_raw[:], cos_sb[:])
                    nc.vector.tensor_mul(tb2[:], sw[:], sin_sb[:])
                    nc.vector.tensor_add(kr_rope[0:DHR, :], ta[:], tb2[:])

                # ---- k^T: l-outer, all 8 (h,sj) accumulators at once ----
                with tc.tile_pool(name="pk8", bufs=1, space="PSUM") as pk8:
                    ps_k = {
                        (h, sj): pk8.tile([P, 512], F32, name=f"ps_k{h}_{sj}")
                        for h in range(HPC) for sj in range(TJ)
                    }
                    for l in range(LKV):
                        for h in range(HPC):
                            for sj in range(TJ):
                                nc.tensor.matmul(
                                    ps_k[(h, sj)][:],
                                    wuk_all[:, l * HPC * HS + h * HS:
                                            l * HPC * HS + (h + 1) * HS],
                                    ckv_t[l][:, sj * 512:(sj + 1) * 512],
                                    start=(l == 0),
                                    stop=(l == LKV - 1),
                                )
                    for h in range(HPC):
                        for sj in range(TJ):
                            nc.vector.tensor_copy(
                                kT[:, h * T + sj * 512: h * T + (sj + 1) * 512],
                                ps_k[(h, sj)][:],
                            )

                # ---- v~ per s-chunk ----
                with tc.tile_pool(name="pv", bufs=3, space="PSUM") as pv:
                    for sc in range(SC):
                        ps = pv.tile([P, HPC * HS], F32, name="ps_v",
                                     tag="psv")
                        for l in range(LKV):
                            nc.tensor.matmul(
                                ps[:],
                                ckv_t[l][:, sc * P:(sc + 1) * P],
                                b_all[:, l * HPC * HS:(l + 1) * HPC * HS],
                                start=(l == 0),
                                stop=(l == LKV - 1),
                            )
                        nc.vector.tensor_copy(
                            v_sb[:, sc * HPC * HS:(sc + 1) * HPC * HS], ps[:]
                        )

                # ---- attention (causal, k-outer, AV one chunk behind) ----
                with (
                    tc.tile_pool(name="psy", bufs=4, space="PSUM") as psy,
                    tc.tile_pool(name="pss", bufs=4, space="PSUM") as pss,
                    tc.tile_pool(name="atp", bufs=12) as atp,
                    tc.tile_pool(name="accp", bufs=6) as accp,
                    tc.tile_pool(name="spool", bufs=3) as spool,
                    tc.tile_pool(name="opool", bufs=3) as opool,
                ):
                    def vslice(k, h):
                        return v_sb[:, k * HPC * HS + h * HS:
                                    k * HPC * HS + (h + 1) * HS]

                    def tail(h, tj, ps_y_t, acc_t):
                        # unnormalized y + denominator out; host divides
                        ps_d = pss.tile([1, 512], F32, name="ps_d",
                                        tag="pss")
                        nc.tensor.matmul(ps_d[:], ones_bf[:], acc_t[:],
                                         start=True, stop=True)
                        den_sb = spool.tile([1, 512], F32, name="den",
                                            tag="den")
                        nc.vector.tensor_copy(den_sb[:], ps_d[:])
                        nc.sync.dma_start(
                            out_den.ap()[h:h + 1, tj * 512:(tj + 1) * 512],
                            den_sb[:],
                        )
                        o_sb = opool.tile([P, 512], F32, name="o_sb", tag="o")
                        nc.vector.tensor_copy(o_sb[:], ps_y_t[:])
                        nc.sync.dma_start(
                            out.ap()[h * HS:(h + 1) * HS,
                                     tj * 512:(tj + 1) * 512],
                            o_sb[:],
                        )

                    snake = []
                    for a, b in zip(range(SC // 2),
                                    range(SC - 1, SC // 2 - 1, -1)):
                        snake += [a, b]
                    last_pos = {
                        tj: max(i for i, k in enumerate(snake)
                                if k <= 4 * tj + 3)
                        for tj in range(TJ)
                    }

                    def off_of(k, tj):
                        return (k - 4 * tj) * P if k // 4 == tj else 0

                    for h in range(HPC):
                        ps_y = {
                            tj: psy.tile([P, 512], F32, name=f"psy{h}_{tj}",
                                         tag="psy")
                            for tj in range(TJ)
                        }
                        acc = {
                            tj: accp.tile([P, 512], BF, name=f"acc{h}_{tj}",
                                          tag="acc")
                            for tj in range(TJ)
                        }
                        pend = {}

                        def emit_av(pos, ps_y=ps_y, acc=acc, pend=pend, h=h):
                            k = snake[pos]
                            for tj, (at_prev, off) in pend.pop(pos).items():
                                nc.tensor.matmul(
                                    ps_y[tj][:, off:], vslice(k, h),
                                    at_prev[:, off:],
                                    start=(pos == 0),
                                    stop=(pos == last_pos[tj]),
                                )
                            for tj in range(TJ):
                                if pos == last_pos[tj]:
                                    tail(h, tj, ps_y[tj], acc[tj])

                        for pos, k in enumerate(snake):
                            tjs = list(range(k // 4, TJ))
                            ats = {}
                            # sub-groups of <=3 so the pss ring can't deadlock
                            for gi in range(0, len(tjs), 3):
                                grp = tjs[gi:gi + 3]
                                ps_t = {}
                                for tj in grp:
                                    off = off_of(k, tj)
                                    ps_s = pss.tile([P, 512], F32,
                                                    name="ps_s", tag="pss")
                                    nc.tensor.matmul(
                                        ps_s[:, off:],
                                        kT[:, h * T + k * P:
                                           h * T + (k + 1) * P],
                                        qT[:, h * T + tj * 512 + off:
                                           h * T + (tj + 1) * 512],
                                        start=True, stop=False,
                                    )
                                    ps_t[tj] = (ps_s, off)
                                for tj in grp:
                                    ps_s, off = ps_t[tj]
                                    nc.tensor.matmul(
                                        ps_s[:, off:],
                                        kr_rope[:, k * P:(k + 1) * P],
                                        qr_rope[:, h * T + tj * 512 + off:
                                                h * T + (tj + 1) * 512],
                                        start=False, stop=True,
                                    )
                                for tj in grp:
                                    ps_s, off = ps_t[tj]
                                    at = atp.tile([P, 512], BF, name="at",
                                                  tag="at")
                                    nc.scalar.activation(
                                        at[:, off:], ps_s[:, off:], Exp,
                                        scale=SCALE,
                                    )
                                    if tj == k // 4:
                                        nc.vector.tensor_mul(
                                            at[:, off:off + P],
                                            at[:, off:off + P],
                                            tri[:],
                                        )
                                    if pos == 0:
                                        nc.vector.tensor_copy(acc[tj][:],
                                                              at[:])
                                    else:
                                        nc.vector.tensor_add(
                                            acc[tj][:, off:],
                                            acc[tj][:, off:],
                                            at[:, off:],
                                        )
                                    ats[tj] = (at, off)
                            pend[pos] = ats
                            if pos - 1 in pend:
                                emit_av(pos - 1)
                        emit_av(len(snake) - 1)
    nc.finalize()
    return nc


_ROPE_PERM = np.concatenate([np.arange(0, DHR, 2), np.arange(1, DHR, 2)])


def _bf(a):
    return np.ascontiguousarray(a).astype(ml_dtypes.bfloat16)


def _prep_inputs(x, freqs_cos, freqs_sin, W_dq, W_uq, W_dkv, W_uk, W_uv, W_qr,
                 W_kr, W_o):
    """Build the 8 per-core input maps (host-side layout prep, all bf16)."""
    x2 = np.asarray(x, np.float32).reshape(T, C)
    xT = np.ascontiguousarray(x2.T)                  # [C, T]
    xT_bf = _bf(xT).reshape(CCH, P, T)
    WdkvT = np.asarray(W_dkv, np.float32).T          # [C, NLKV]
    wkrT = np.asarray(W_kr, np.float32)[_ROPE_PERM, :].T   # [C, DHR]
    cosT = np.asarray(freqs_cos, np.float32).T       # [32, T]
    sinT = np.asarray(freqs_sin, np.float32).T
    cos2T = _bf(np.concatenate([cosT, cosT], axis=0))    # [64, T]
    sin2T = _bf(np.concatenate([-sinT, sinT], axis=0))

    Wdq = np.asarray(W_dq, np.float32)               # [NLQ, C]
    Wuq_mat = np.asarray(W_uq, np.float32).reshape(NLQ, NH * HS)
    Wq_comb = Wdq.T @ Wuq_mat                        # [C, NH*HS]
    Wqr_comb = Wdq.T @ np.asarray(W_qr, np.float32).T    # [C, NH*DHR]
    v_eff = np.asarray(W_uv, np.float32).T @ np.asarray(W_o, np.float32).T
    W_uk_a = np.asarray(W_uk)

    wkrP = _bf(wkrT.reshape(CCH, P, DHR).transpose(1, 0, 2)
               .reshape(P, CCH * DHR))

    in_maps = []
    for i in range(NCORES):
        h0 = i * HPC
        cols = slice(h0 * HS, (h0 + HPC) * HS)       # 256 output cols
        wqr_cols = np.concatenate(
            [Wqr_comb[:, (h0 + h) * DHR + _ROPE_PERM] for h in range(HPC)],
            axis=1,
        )                                            # [C, HPC*64=128]
        hi = i % 2                                   # nlkv-row half
        tb = i // 2                                  # 512-token block
        xs_sl = xT[:, tb * 512:(tb + 1) * 512]       # [C, 512]
        in_maps.append({
            "xTp": xT_bf,
            "xsP": _bf(xs_sl.reshape(CCH, P, 512).transpose(1, 0, 2)
                       .reshape(P, CCH * 512)),
            "wdkvP": _bf(WdkvT[:, hi * 256:(hi + 1) * 256]
                         .reshape(CCH, P, 256).transpose(1, 0, 2)
                         .reshape(P, CCH * 256)),
            "wkrP": wkrP,
            "cos2T": cos2T,
            "sin2T": sin2T,
            "wq": _bf(Wq_comb[:, cols]).reshape(CCH, P, HPC * HS),
            "wqr": _bf(wqr_cols).reshape(CCH, P, HPC * DHR),
            "wukT": _bf(np.ascontiguousarray(
                        W_uk_a[h0 * HS:(h0 + HPC) * HS, :].T)
                        .reshape(LKV, P, HPC * HS)),
            "bc": _bf(v_eff[:, cols]).reshape(LKV, P, HPC * HS),
        })
    return in_maps


def _finish(res):
    """Divide by den on the host and assemble the full [B,T,C] output."""
    outs = []
    for i in range(NCORES):
        y = np.asarray(res.results[i]["out"], np.float32)      # [HPC*HS, T]
        den = np.asarray(res.results[i]["out_den"], np.float32)  # [HPC, T]
        y = y.reshape(HPC, HS, T) / den[:, None, :]
        outs.append(y.transpose(2, 0, 1).reshape(T, HPC * HS))
    return np.concatenate(outs, axis=1).reshape(B, T, C)


_NC_CACHE = None


def kernel(**inputs):
    global _NC_CACHE
    in_maps = _prep_inputs(**inputs)
    if _NC_CACHE is None:
        _NC_CACHE = build_nc()
    res = run_bass_kernel_spmd(_NC_CACHE, in_maps, core_ids=list(range(NCORES)))
    return _finish(res)


# revision 27
# speedup vs baseline: 1.0332x; 1.0332x over previous
"""MLA-style attention (nn_Attention_7868380086611) on 8 TRN2 NeuronCores.

Strategy (v7)
-------------
Head-parallel attention (2 of 16 heads per core), query path fully absorbed
on the host (W_dq.T @ W_uq / W_dq.T @ W_qr.T), tiny shared kv latent
(c_kv 512 rows + k_r 64 rows) computed T-sharded and AllGathered once.

v7 over the 261us v3 baseline:
- Phase-1 inputs host-prepacked into SBUF-image layout: 9 simple 2D DMAs
  (cheap descriptor gen) interleaved so the c-outer 3-accumulator matmul
  chain starts as soon as the first quarter lands.  cc_in stores go on the
  SYNC queue *before* the 8MB xT loads (descriptor rings drain FIFO), so
  the AllGather triggers at ~26us instead of ~42us.
- Rope operands zero-padded 64->128 partitions (K=64 matmuls measured
  369ns vs 260ns for K=128).
- Causal diagonal-suffix: diagonal score blocks only compute the unmasked
  column suffix for score/rope/AV matmuls, exp, mask and accumulate.
- kT is l-outer (8 PSUM accumulators at once) right after the gather.
- Softmax denominator division done on the HOST (kernel returns
  unnormalized y [HPC*HS, T] + den [HPC, T]): kills all den/yT transposes.
- PSUM->SBUF projection copies on Vector (Scalar = exp critical path).
"""

import math
import sys

import numpy as np

sys.path.insert(0, "/opt/trn_rl_repo")

import ml_dtypes  # noqa: E402

from concourse import bacc, bass, masks, mybir  # noqa: E402
from concourse.bass_utils import run_bass_kernel_spmd  # noqa: E402
from concourse.tile import TileContext  # noqa: E402

B, T, C = 1, 2048, 2048
NH, HS = 16, 128
NLQ, NLKV, DHR = 1536, 512, 64
NCORES = 8
HPC = NH // NCORES          # heads per core = 2
TS = T // NCORES            # 256-token shard for the kv down-projection
P = 128
LKV = NLKV // P             # 4
CCH = C // P                # 16 c-chunks
TJ = T // 512               # 4 t-chunks of 512
SC = T // P                 # 16 s-chunks
SCALE = 1.0 / math.sqrt(HS + DHR)

BF = mybir.dt.bfloat16
F32 = mybir.dt.float32
Exp = mybir.ActivationFunctionType.Exp

GR = NLKV // 2 + DHR        # 320 rows in the all-gather buffer
TS2 = 2 * TS                # 512-token slice


def build_nc():
    nc = bacc.Bacc(None, target_bir_lowering=False, num_devices=NCORES)

    xTp = nc.declare_dram_parameter("xTp", [CCH, P, T], BF, isOutput=False)
    wdkvP = nc.declare_dram_parameter("wdkvP", [P, CCH * 256], BF,
                                      isOutput=False)
    wkrP = nc.declare_dram_parameter("wkrP", [P, CCH * DHR], BF,
                                     isOutput=False)
    cos2T = nc.declare_dram_parameter("cos2T", [DHR, T], BF, isOutput=False)
    sin2T = nc.declare_dram_parameter("sin2T", [DHR, T], BF, isOutput=False)
    wq = nc.declare_dram_parameter("wq", [CCH, P, HPC * HS], BF, isOutput=False)
    wqr = nc.declare_dram_parameter("wqr", [CCH, P, HPC * DHR], BF, isOutput=False)
    wukT = nc.declare_dram_parameter("wukT", [LKV, P, HPC * HS], BF, isOutput=False)
    bc = nc.declare_dram_parameter("bc", [LKV, P, HPC * HS], BF, isOutput=False)
    xsP = nc.declare_dram_parameter("xsP", [P, CCH * TS2], BF, isOutput=False)
    out = nc.declare_dram_parameter("out", [HPC * HS, T], F32, isOutput=True)
    out_den = nc.declare_dram_parameter("out_den", [HPC, T], F32, isOutput=True)

    cc_in_kv = nc.dram_tensor("cc_in_kv", [GR, TS2], BF)
    cc_out_kv = nc.dram_tensor("cc_out_kv", [NCORES, GR, TS2], BF,
                               addr_space="Shared")

    with TileContext(nc) as tc:
        with (
            tc.tile_pool(name="persist", bufs=1) as persist,
            tc.tile_pool(name="lat", bufs=1) as lat,
            tc.tile_pool(name="proj", bufs=1) as proj,
            tc.tile_pool(name="wts", bufs=1) as wts,
        ):
            # ---- constants (gpsimd: free early) ----
            junk = persist.tile([P, 512], BF)
            nc.gpsimd.memset(junk[:], 0.0)
            ones_bf = persist.tile([P, 1], BF)
            nc.gpsimd.memset(ones_bf[:], 1.0)
            # single [128,128] triangular mask: 1 iff t - s >= 0
            tri = persist.tile([P, P], BF)
            nc.gpsimd.memset(tri[:], 1.0)
            nc.gpsimd.affine_select(
                out=tri[:], in_=tri[:],
                compare_op=mybir.AluOpType.is_ge,
                fill=0.0, base=0, channel_multiplier=-1,
                pattern=[[1, P]],
            )
            cos_sb = persist.tile([DHR, T], BF)
            sin_sb = persist.tile([DHR, T], BF)

            # rope-padded tiles: rows 64-127 stay zero (K=128 matmuls)
            qr_rope = proj.tile([P, HPC * T], BF)
            kr_rope = proj.tile([P, T], BF)
            qr_h1 = proj.tile([P, T], BF)
            nc.vector.memset(qr_rope[:], 0.0)
            nc.vector.memset(kr_rope[:], 0.0)
            nc.vector.memset(qr_h1[:], 0.0)

            qT = proj.tile([P, HPC * T], BF)
            kT = proj.tile([P, HPC * T], BF)
            qr2 = proj.tile([P, T], BF)
            v_sb = proj.tile([P, SC * HPC * HS], BF)

            # ---- phase 1: c_kv^T/k_r^T for own slice -> AllGather ----
            with (
                tc.tile_pool(name="p1w", bufs=1) as p1w,
                tc.tile_pool(name="p1ps", bufs=1, space="PSUM") as p1ps,
                tc.tile_pool(name="p1sh", bufs=1) as p1sh,
            ):
                # PE warm-up while the first DMA quarters land (~3us)
                ps_w = p1ps.tile([P, 512], F32, name="ps_warm")
                for _ in range(5):
                    nc.tensor.matmul(ps_w[:], junk[:, 0:P], junk[:],
                                     start=True, stop=True)

                # interleaved quarter loads (host-prepacked, simple 2D
                # DMAs); per-quarter tiles so the c=0 matmuls only wait on
                # quarter 0, not the whole image
                wdkv_q = [p1w.tile([P, 1024], BF, name=f"wdkv{g}",
                                   tag=f"wdkv{g}") for g in range(4)]
                xs_q = [p1w.tile([P, 2048], BF, name=f"xs{g}",
                                 tag=f"xs{g}") for g in range(4)]
                wkr_sb = p1w.tile([P, CCH * DHR], BF, name="wkr_sb")
                for g in range(4):
                    nc.sync.dma_start(
                        wdkv_q[g][:], wdkvP.ap()[:, g * 1024:(g + 1) * 1024]
                    )
                    nc.sync.dma_start(
                        xs_q[g][:], xsP.ap()[:, g * 2048:(g + 1) * 2048]
                    )
                    if g == 0:
                        nc.sync.dma_start(wkr_sb[:], wkrP.ap())
                wqr_all = wts.tile([P, CCH * HPC * DHR], BF)
                nc.sync.dma_start(
                    wqr_all[:].rearrange("p (n m) -> p n m", n=CCH),
                    wqr.ap().rearrange("n p m -> p n m"),
                )

                def xstile(c):
                    return xs_q[c // 4][:, (c % 4) * TS2:(c % 4 + 1) * TS2]

                def wdkv_sl(c, ls):
                    return wdkv_q[c // 4][:, (c % 4) * 256 + ls * P:
                                          (c % 4) * 256 + (ls + 1) * P]

                ps0 = p1ps.tile([P, TS2], F32, name="ps0")
                ps1 = p1ps.tile([P, TS2], F32, name="ps1")
                pskr = p1ps.tile([DHR, TS2], F32, name="pskr")
                for c in range(CCH):
                    st, sp = (c == 0), (c == CCH - 1)
                    nc.tensor.matmul(ps0[:], wdkv_sl(c, 0), xstile(c),
                                     start=st, stop=sp)
                    nc.tensor.matmul(ps1[:], wdkv_sl(c, 1), xstile(c),
                                     start=st, stop=sp)
                    nc.tensor.matmul(pskr[:],
                                     wkr_sb[:, c * DHR:(c + 1) * DHR],
                                     xstile(c), start=st, stop=sp)
                sh0 = p1sh.tile([P, TS2], BF, name="sh0")
                sh1 = p1sh.tile([P, TS2], BF, name="sh1")
                shkr = p1sh.tile([DHR, TS2], BF, name="shkr")
                nc.scalar.copy(sh0[:], ps0[:])
                nc.vector.tensor_copy(sh1[:], ps1[:])
                nc.scalar.copy(shkr[:], pskr[:])
                # stores on the SYNC queue, BEFORE the big xT loads are
                # enqueued -> they reach the FIFO DMA rings first and the
                # AllGather triggers ~16us earlier.
                nc.sync.dma_start(cc_in_kv[0:P, :], sh0[:])
                nc.sync.dma_start(cc_in_kv[P:2 * P, :], sh1[:])
                nc.sync.dma_start(cc_in_kv[2 * P:GR, :], shkr[:])

                nc.gpsimd.collective_compute(
                    "AllGather",
                    mybir.AluOpType.bypass,
                    replica_groups=[list(range(NCORES))],
                    ins=[cc_in_kv.ap().opt()],
                    outs=[cc_out_kv.ap().opt()],
                )

                # keep the HAM clock gate open across the xT-load bubble
                for _ in range(6):
                    nc.tensor.matmul(ps_w[:], junk[:, 0:P], junk[:],
                                     start=True, stop=True)

            nc.scalar.dma_start(cos_sb[:], cos2T[:, :])
            nc.scalar.dma_start(sin_sb[:], sin2T[:, :])

            # ---- remaining loads (sync queue, after the cc stores) ----
            wq_all = wts.tile([P, CCH * HPC * HS], BF)
            nc.sync.dma_start(
                wq_all[:].rearrange("p (n m) -> p n m", n=CCH),
                wq.ap().rearrange("n p m -> p n m"),
            )
            xt = []
            for cgrp in range(4):
                t = lat.tile([P, 4 * T], BF, name=f"xt{cgrp}", tag=f"xt{cgrp}")
                for j in range(4):
                    nc.sync.dma_start(
                        t[:, j * T:(j + 1) * T], xTp.ap()[4 * cgrp + j]
                    )
                xt.append(t)

            def xtile(c):
                return xt[c // 4][:, (c % 4) * T:(c % 4 + 1) * T]

            wuk_all = wts.tile([P, LKV * HPC * HS], BF)
            nc.sync.dma_start(
                wuk_all[:].rearrange("p (n m) -> p n m", n=LKV),
                wukT.ap().rearrange("n p m -> p n m"),
            )
            b_all = wts.tile([P, LKV * HPC * HS], BF)
            nc.sync.dma_start(
                b_all[:].rearrange("p (n m) -> p n m", n=LKV),
                bc.ap().rearrange("n p m -> p n m"),
            )

            with tc.tile_pool(name="rtmp", bufs=1) as rtmp:

                def rope(dst, src):
                    # dst = src * [cos;cos] + swap_halves(src) * [-sin;sin]
                    sw = rtmp.tile([DHR, T], BF, name="rsw", tag="rsw")
                    nc.sync.dma_start(sw[0:32, :], src[32:64, :])
                    nc.sync.dma_start(sw[32:64, :], src[0:32, :])
                    ta = rtmp.tile([DHR, T], BF, name="rta", tag="rta")
                    tb = rtmp.tile([DHR, T], BF, name="rtb", tag="rtb")
                    nc.vector.tensor_mul(ta[:], src, cos_sb[:])
                    nc.vector.tensor_mul(tb[:], sw[:], sin_sb[:])
                    nc.vector.tensor_add(dst, ta[:], tb[:])

                with tc.tile_pool(name="p5ps", bufs=5, space="PSUM") as p5ps:
                    # q_r^T both heads in one pass (M=128), tj-inner
                    ps_qr = [
                        p5ps.tile([P, 512], F32, name=f"ps_qr{tj}", tag="p5")
                        for tj in range(TJ)
                    ]
                    for c in range(CCH):
                        for tj in range(TJ):
                            nc.tensor.matmul(
                                ps_qr[tj][:],
                                wqr_all[:, c * HPC * DHR:(c + 1) * HPC * DHR],
                                xtile(c)[:, tj * 512:(tj + 1) * 512],
                                start=(c == 0),
                                stop=(c == CCH - 1),
                            )
                    for tj in range(TJ):
                        nc.vector.tensor_copy(
                            qr2[:, tj * 512:(tj + 1) * 512], ps_qr[tj][:]
                        )
                    nc.sync.dma_start(qr_h1[0:DHR, :], qr2[DHR:P, :])
                    rope(qr_rope[0:DHR, 0:T], qr2[0:DHR, :])
                    rope(qr_rope[0:DHR, T:HPC * T], qr_h1[0:DHR, :])

                    # q^T per head, tj-inner
                    for h in range(HPC):
                        ps_q = [
                            p5ps.tile([P, 512], F32, name=f"ps_q{h}_{tj}",
                                      tag="p5")
                            for tj in range(TJ)
                        ]
                        for c in range(CCH):
                            for tj in range(TJ):
                                nc.tensor.matmul(
                                    ps_q[tj][:],
                                    wq_all[:, c * HPC * HS + h * HS:
                                           c * HPC * HS + (h + 1) * HS],
                                    xtile(c)[:, tj * 512:(tj + 1) * 512],
                                    start=(c == 0),
                                    stop=(c == CCH - 1),
                                )
                        for tj in range(TJ):
                            nc.vector.tensor_copy(
                                qT[:, h * T + tj * 512: h * T + (tj + 1) * 512],
                                ps_q[tj][:],
                            )

                # ---- gathered kv latents (rank r = 2*tb+hi holds nlkv-half
                # hi of token slice tb; kr lives on the even-rank halves) ----
                cc_halves = cc_out_kv.ap().rearrange(
                    "(b two) r u -> two b r u", two=2
                )
                ckv_t = []
                for l in range(LKV):
                    tl = lat.tile([P, T], BF, name=f"ckv{l}", tag=f"ckv{l}")
                    nc.sync.dma_start(
                        tl[:].rearrange("p (g u) -> p g u", g=4),
                        cc_halves[l // 2]
                        [:, (l % 2) * P:(l % 2 + 1) * P, :].rearrange(
                            "g p u -> p g u"
                        ),
                    )
                    ckv_t.append(tl)
                kr_raw = lat.tile([DHR, T], BF)
                nc.sync.dma_start(
                    kr_raw[:].rearrange("p (g u) -> p g u", g=4),
                    cc_halves[0][:, 2 * P:GR, :].rearrange("g p u -> p g u"),
                )
                with tc.tile_pool(name="rtmp2", bufs=1) as rtmp2:
                    sw = rtmp2.tile([DHR, T], BF, name="krsw")
                    nc.sync.dma_start(sw[0:32, :], kr_raw[32:64, :])
                    nc.sync.dma_start(sw[32:64, :], kr_raw[0:32, :])
                    ta = rtmp2.tile([DHR, T], BF, name="krta")
                    tb2 = rtmp2.tile([DHR, T], BF, name="krtb")
                    nc.vector.tensor_mul(ta[:], kr_raw[:], cos_sb[:])
                    nc.vector.tensor_mul(tb2[:], sw[:], sin_sb[:])
                    nc.vector.tensor_add(kr_rope[0:DHR, :], ta[:], tb2[:])

                # ---- k^T: l-outer, all 8 (h,sj) accumulators at once ----
                with tc.tile_pool(name="pk8", bufs=1, space="PSUM") as pk8:
                    ps_k = {
                        (h, sj): pk8.tile([P, 512], F32, name=f"ps_k{h}_{sj}")
                        for h in range(HPC) for sj in range(TJ)
                    }
                    for l in range(LKV):
                        for h in range(HPC):
                            for sj in range(TJ):
                                nc.tensor.matmul(
                                    ps_k[(h, sj)][:],
                                    wuk_all[:, l * HPC * HS + h * HS:
                                            l * HPC * HS + (h + 1) * HS],
                                    ckv_t[l][:, sj * 512:(sj + 1) * 512],
                                    start=(l == 0),
                                    stop=(l == LKV - 1),
                                )
                    for h in range(HPC):
                        for sj in range(TJ):
                            nc.vector.tensor_copy(
                                kT[:, h * T + sj * 512: h * T + (sj + 1) * 512],
                                ps_k[(h, sj)][:],
                            )

                # ---- v~ per s-chunk ----
                with tc.tile_pool(name="pv", bufs=3, space="PSUM") as pv:
                    for sc in range(SC):
                        ps = pv.tile([P, HPC * HS], F32, name="ps_v",
                                     tag="psv")
                        for l in range(LKV):
                            nc.tensor.matmul(
                                ps[:],
                                ckv_t[l][:, sc * P:(sc + 1) * P],
                                b_all[:, l * HPC * HS:(l + 1) * HPC * HS],
                                start=(l == 0),
                                stop=(l == LKV - 1),
                            )
                        nc.vector.tensor_copy(
                            v_sb[:, sc * HPC * HS:(sc + 1) * HPC * HS], ps[:]
                        )

                # ---- attention (causal, k-outer, AV one chunk behind) ----
                with (
                    tc.tile_pool(name="psy", bufs=4, space="PSUM") as psy,
                    tc.tile_pool(name="pss", bufs=4, space="PSUM") as pss,
                    tc.tile_pool(name="atp", bufs=12) as atp,
                    tc.tile_pool(name="accp", bufs=6) as accp,
                    tc.tile_pool(name="spool", bufs=3) as spool,
                    tc.tile_pool(name="opool", bufs=3) as opool,
                ):
                    def vslice(k, h):
                        return v_sb[:, k * HPC * HS + h * HS:
                                    k * HPC * HS + (h + 1) * HS]

                    def tail(h, tj, ps_y_t, acc_t):
                        # unnormalized y + denominator out; host divides
                        ps_d = pss.tile([1, 512], F32, name="ps_d",
                                        tag="pss")
                        nc.tensor.matmul(ps_d[:], ones_bf[:], acc_t[:],
                                         start=True, stop=True)
                        den_sb = spool.tile([1, 512], F32, name="den",
                                            tag="den")
                        nc.vector.tensor_copy(den_sb[:], ps_d[:])
                        nc.sync.dma_start(
                            out_den.ap()[h:h + 1, tj * 512:(tj + 1) * 512],
                            den_sb[:],
                        )
                        o_sb = opool.tile([P, 512], F32, name="o_sb", tag="o")
                        nc.vector.tensor_copy(o_sb[:], ps_y_t[:])
                        nc.sync.dma_start(
                            out.ap()[h * HS:(h + 1) * HS,
                                     tj * 512:(tj + 1) * 512],
                            o_sb[:],
                        )

                    snake = []
                    for a, b in zip(range(SC // 2),
                                    range(SC - 1, SC // 2 - 1, -1)):
                        snake += [a, b]
                    last_pos = {
                        tj: max(i for i, k in enumerate(snake)
                                if k <= 4 * tj + 3)
                        for tj in range(TJ)
                    }

                    def off_of(k, tj):
                        return (k - 4 * tj) * P if k // 4 == tj else 0

                    for h in range(HPC):
                        ps_y = {
                            tj: psy.tile([P, 512], F32, name=f"psy{h}_{tj}",
                                         tag="psy")
                            for tj in range(TJ)
                        }
                        acc = {
                            tj: accp.tile([P, 512], BF, name=f"acc{h}_{tj}",
                                          tag="acc")
                            for tj in range(TJ)
                        }
                        pend = {}

                        def emit_av(pos, ps_y=ps_y, acc=acc, pend=pend, h=h):
                            k = snake[pos]
                            for tj, (at_prev, off) in pend.pop(pos).items():
                                nc.tensor.matmul(
                                    ps_y[tj][:, off:], vslice(k, h),
                                    at_prev[:, off:],
                                    start=(pos == 0),
                                    stop=(pos == last_pos[tj]),
                                )
                            for tj in range(TJ):
                                if pos == last_pos[tj]:
                                    tail(h, tj, ps_y[tj], acc[tj])

                        for pos, k in enumerate(snake):
                            tjs = list(range(k // 4, TJ))
                            ats = {}
                            # sub-groups of <=3 so the pss ring can't deadlock
                            for gi in range(0, len(tjs), 3):
                                grp = tjs[gi:gi + 3]
                                # per-tj [content, rope, exp] interleave:
                                # each exp starts two matmuls earlier, so
                                # the pss ring drains at production rate
                                for tj in grp:
                                    off = off_of(k, tj)
                                    ps_s = pss.tile([P, 512], F32,
                                                    name="ps_s", tag="pss")
                                    nc.tensor.matmul(
                                        ps_s[:, off:],
                                        kT[:, h * T + k * P:
                                           h * T + (k + 1) * P],
                                        qT[:, h * T + tj * 512 + off:
                                           h * T + (tj + 1) * 512],
                                        start=True, stop=False,
                                    )
                                    nc.tensor.matmul(
                                        ps_s[:, off:],
                                        kr_rope[:, k * P:(k + 1) * P],
                                        qr_rope[:, h * T + tj * 512 + off:
                                                h * T + (tj + 1) * 512],
                                        start=False, stop=True,
                                    )
                                    at = atp.tile([P, 512], BF, name="at",
                                                  tag="at")
                                    nc.scalar.activation(
                                        at[:, off:], ps_s[:, off:], Exp,
                                        scale=SCALE,
                                    )
                                    if tj == k // 4:
                                        nc.vector.tensor_mul(
                                            at[:, off:off + P],
                                            at[:, off:off + P],
                                            tri[:],
                                        )
                                    if pos == 0:
                                        nc.vector.tensor_copy(acc[tj][:],
                                                              at[:])
                                    else:
                                        nc.vector.tensor_add(
                                            acc[tj][:, off:],
                                            acc[tj][:, off:],
                                            at[:, off:],
                                        )
                                    ats[tj] = (at, off)
                            pend[pos] = ats
                            if pos - 1 in pend:
                                emit_av(pos - 1)
                        emit_av(len(snake) - 1)
    nc.finalize()
    return nc


_ROPE_PERM = np.concatenate([np.arange(0, DHR, 2), np.arange(1, DHR, 2)])


def _bf(a):
    return np.ascontiguousarray(a).astype(ml_dtypes.bfloat16)


def _prep_inputs(x, freqs_cos, freqs_sin, W_dq, W_uq, W_dkv, W_uk, W_uv, W_qr,
                 W_kr, W_o):
    """Build the 8 per-core input maps (host-side layout prep, all bf16)."""
    x2 = np.asarray(x, np.float32).reshape(T, C)
    xT = np.ascontiguousarray(x2.T)                  # [C, T]
    xT_bf = _bf(xT).reshape(CCH, P, T)
    WdkvT = np.asarray(W_dkv, np.float32).T          # [C, NLKV]
    wkrT = np.asarray(W_kr, np.float32)[_ROPE_PERM, :].T   # [C, DHR]
    cosT = np.asarray(freqs_cos, np.float32).T       # [32, T]
    sinT = np.asarray(freqs_sin, np.float32).T
    cos2T = _bf(np.concatenate([cosT, cosT], axis=0))    # [64, T]
    sin2T = _bf(np.concatenate([-sinT, sinT], axis=0))

    Wdq = np.asarray(W_dq, np.float32)               # [NLQ, C]
    Wuq_mat = np.asarray(W_uq, np.float32).reshape(NLQ, NH * HS)
    Wq_comb = Wdq.T @ Wuq_mat                        # [C, NH*HS]
    Wqr_comb = Wdq.T @ np.asarray(W_qr, np.float32).T    # [C, NH*DHR]
    v_eff = np.asarray(W_uv, np.float32).T @ np.asarray(W_o, np.float32).T
    W_uk_a = np.asarray(W_uk)

    wkrP = _bf(wkrT.reshape(CCH, P, DHR).transpose(1, 0, 2)
               .reshape(P, CCH * DHR))

    in_maps = []
    for i in range(NCORES):
        h0 = i * HPC
        cols = slice(h0 * HS, (h0 + HPC) * HS)       # 256 output cols
        wqr_cols = np.concatenate(
            [Wqr_comb[:, (h0 + h) * DHR + _ROPE_PERM] for h in range(HPC)],
            axis=1,
        )                                            # [C, HPC*64=128]
        hi = i % 2                                   # nlkv-row half
        tb = i // 2                                  # 512-token block
        xs_sl = xT[:, tb * 512:(tb + 1) * 512]       # [C, 512]
        in_maps.append({
            "xTp": xT_bf,
            "xsP": _bf(xs_sl.reshape(CCH, P, 512).transpose(1, 0, 2)
                       .reshape(P, CCH * 512)),
            "wdkvP": _bf(WdkvT[:, hi * 256:(hi + 1) * 256]
                         .reshape(CCH, P, 256).transpose(1, 0, 2)
                         .reshape(P, CCH * 256)),
            "wkrP": wkrP,
            "cos2T": cos2T,
            "sin2T": sin2T,
            "wq": _bf(Wq_comb[:, cols]).reshape(CCH, P, HPC * HS),
            "wqr": _bf(wqr_cols).reshape(CCH, P, HPC * DHR),
            "wukT": _bf(np.ascontiguousarray(
                        W_uk_a[h0 * HS:(h0 + HPC) * HS, :].T)
                        .reshape(LKV, P, HPC * HS)),
            "bc": _bf(v_eff[:, cols]).reshape(LKV, P, HPC * HS),
        })
    return in_maps


def _finish(res):
    """Divide by den on the host and assemble the full [B,T,C] output."""
    outs = []
    for i in range(NCORES):
        y = np.asarray(res.results[i]["out"], np.float32)      # [HPC*HS, T]
        den = np.asarray(res.results[i]["out_den"], np.float32)  # [HPC, T]
        y = y.reshape(HPC, HS, T) / den[:, None, :]
        outs.append(y.transpose(2, 0, 1).reshape(T, HPC * HS))
    return np.concatenate(outs, axis=1).reshape(B, T, C)


_NC_CACHE = None


def kernel(**inputs):
    global _NC_CACHE
    in_maps = _prep_inputs(**inputs)
    if _NC_CACHE is None:
        _NC_CACHE = build_nc()
    res = run_bass_kernel_spmd(_NC_CACHE, in_maps, core_ids=list(range(NCORES)))
    return _finish(res)
